# revision 1
# baseline (speedup 1.0000x reference)
"""Trainium2 Bass kernel for nn_CBAM_84799834292534.

Strategy:
- 8 cores = 4 batch samples x 2 vertical halves. Half-1 cores receive
  row-flipped inputs/weights so every core runs the identical program
  ("local top" = its outer image edge, halo rows toward the cut edge).
- Halos handled by redundant compute (no halo exchange).
- SFOM's DCT gating collapses analytically: idct(gate*dct(x)) == gate*x,
  and mean(dct(x)) == dot(x, w) with w = idct_ortho(ones)/N.
- Convs are shifted matmuls with channels on partitions, f32r dtype.
  K-packing (stacking dj-shifts along the contraction dim via shifted
  input copies) and M-packing (stacking dj-shifts along output channels,
  combined with shifted PSUM adds) keep the PE near full utilization.
- InstanceNorm/BatchNorm/DCT-mean stats use partial sums + tiny
  AllReduces (pair groups for per-sample stats, all-8 for BatchNorm).
  Stat bands are computed first so the AllReduce overlaps halo tiles.
"""
import sys
import types

sys.path.insert(0, '/opt/trn_rl_repo')
import numpy as np

B, C, H0, W0 = 4, 128, 128, 128
H1 = 130          # after conv1
H2 = 132          # after conv2 (final spatial)
ST = 138          # uniform row stride of on-chip layouts
EPS = 1e-5
NCORES = 8
HB = 66           # output band rows per core

XROWS = 84        # x rows needed per core
R_F = 84          # f (SFOM output) data rows
R_H = [81, 78, 75, 72, 69, 66]   # SPEM layer output rows (local)

CNT1_LOC = 65 * 130
CNT1_TOT = 130 * 130
CNT2_LOC = 66 * 132
CNT2_TOT = 132 * 132
CNTB_LOC = 66 * 132
CNTB_TOT = 8 * 66 * 132


def _idct_ortho_np(Xin):
    """numpy copy of the reference _idct_ortho (float64)."""
    X = np.asarray(Xin, np.float64)
    N = X.shape[-1]
    scale = np.full(N, np.sqrt(N / 2.0))
    scale[0] = np.sqrt(float(N))
    Xv = X * scale
    k = np.arange(N) * (np.pi / (2.0 * N))
    Wr, Wi = np.cos(k), np.sin(k)
    Vti = np.concatenate([np.zeros(1), -Xv[::-1][:-1]])
    V = (Xv * Wr - Vti * Wi) + 1j * (Xv * Wi + Vti * Wr)
    v = np.fft.ifft(V).real
    out = np.zeros_like(v)
    out[0::2] = v[: (N + 1) // 2]
    out[1::2] = v[::-1][: N // 2]
    return out


def dct_mean_weights():
    """w such that mean(dct_ortho(x)) == dot(x, w), x of length H2*W2."""
    N = H2 * H2
    return _idct_ortho_np(np.ones(N)) / N


def _install_ntff_hook():
    if "antenv.axon_hooks" in sys.modules:
        return
    mod = types.ModuleType("antenv.axon_hooks")
    _state = {"hook": None}
    mod.set_axon_ntff_profile_hook = lambda h: _state.__setitem__("hook", h)
    mod.get_axon_ntff_profile_hook = lambda: _state["hook"]
    sys.modules["antenv.axon_hooks"] = mod
    try:
        from trn_agent_boot.trn_boot import _ntff_profile_via_ctypes
        mod.set_axon_ntff_profile_hook(
            _ntff_profile_via_ctypes('/opt/axon/libaxon_pjrt.so'))
    except Exception:
        pass


# ----------------------------------------------------------------------------
# program build
# ----------------------------------------------------------------------------

_PROGRAM_CACHE = {}


class _StopBuild(Exception):
    pass


def build_program(debug_taps=False, stage_limit=99):
    key = (bool(debug_taps), stage_limit)
    if key in _PROGRAM_CACHE:
        return _PROGRAM_CACHE[key]

    import concourse.bacc as bacc
    import concourse.tile as tile
    from concourse import mybir

    f32 = mybir.dt.float32
    f32r = mybir.dt.float32r
    AF = mybir.ActivationFunctionType
    AL = mybir.AluOpType
    AX = mybir.AxisListType

    nc = bacc.Bacc("TRN2", target_bir_lowering=False)

    # ---------------- external tensors ----------------
    xs_d = nc.dram_tensor("xs", [C, XROWS, W0], f32, kind="ExternalInput")
    w1_d = nc.dram_tensor("w1", [C, 9, C], f32r, kind="ExternalInput")
    w2_d = nc.dram_tensor("w2c", [C, 9, C], f32r, kind="ExternalInput")
    s_d = [
        nc.dram_tensor("s1", [C, 14, 128], f32r, kind="ExternalInput"),
        nc.dram_tensor("s2", [C, 7, 128], f32r, kind="ExternalInput"),
        nc.dram_tensor("s3", [C, 28, 128], f32r, kind="ExternalInput"),
        nc.dram_tensor("s4", [C, 49, 128], f32r, kind="ExternalInput"),
        nc.dram_tensor("s5", [C, 28, 128], f32r, kind="ExternalInput"),
        nc.dram_tensor("s6", [C, 14, 128], f32r, kind="ExternalInput"),
    ]
    cw_d = nc.dram_tensor("cw", [32, 1], f32, kind="ExternalInput")
    cb_d = nc.dram_tensor("cb", [1, 1], f32, kind="ExternalInput")
    sa1_d = nc.dram_tensor("sa1t", [C, 8], f32, kind="ExternalInput")
    sa2_d = nc.dram_tensor("sa2t", [8, C], f32, kind="ExternalInput")
    gb_d = nc.dram_tensor("gb", [6, 2, C], f32, kind="ExternalInput")
    wv_d = nc.dram_tensor("wv", [HB, H2], f32, kind="ExternalInput")
    out_d = nc.dram_tensor("out", [C, HB, H2], f32, kind="ExternalOutput")
    S_W = float(dct_mean_weights().sum())

    taps = {}
    if debug_taps:
        def tap(name, shape):
            taps[name] = nc.dram_tensor("tap_" + name, shape, f32,
                                        kind="ExternalOutput")
        tap("xp", [C, 87, ST])
        tap("out1", [C, 84, ST])
        tap("r1p", [C, 88, ST])
        tap("out2", [C, 84, ST])
        tap("f", [C, 89, ST])
        tap("stats1", [C, 2])
        tap("stats2", [C, 3])
        tap("gate", [C, 1])
        for k in range(6):
            tap(f"h{k+1}", [128 if k < 5 else 32, R_H[k] + 5, ST])

    PAIRS = [[0, 1], [2, 3], [4, 5], [6, 7]]
    ALL8 = [list(range(NCORES))]

    with tile.TileContext(nc) as tc:
        stage = tc.alloc_tile_pool(name="stage", bufs=2)
        fpool = tc.alloc_tile_pool(name="fpool", bufs=1)
        wts = tc.alloc_tile_pool(name="wts", bufs=1)
        cons = tc.alloc_tile_pool(name="cons", bufs=1)
        sm = tc.alloc_tile_pool(name="sm", bufs=2)
        smc = tc.alloc_tile_pool(name="smc", bufs=1)
        wvp = tc.alloc_tile_pool(name="wvp", bufs=2)
        otp = tc.alloc_tile_pool(name="otp", bufs=2)
        sap = tc.alloc_tile_pool(name="sap", bufs=3)
        cps = tc.alloc_tile_pool(name="cps", bufs=6, space="PSUM")
        mps = tc.alloc_tile_pool(name="mps", bufs=2, space="PSUM")
        drp = tc.alloc_tile_pool(name="drp", bufs=1, space="DRAM")

        def flat(t):
            return t.rearrange("p r c -> p (r c)")

        def ckpt(n):
            if stage_limit <= n:
                raise _StopBuild()

        # ---------------- constants ----------------
        sa1_sb = cons.tile([C, 8], f32, tag="sa1")
        sa2_sb = cons.tile([8, C], f32, tag="sa2")
        cw_sb = cons.tile([32, 1], f32, tag="cw")
        cb_sb = cons.tile([1, 1], f32, tag="cb")
        gb_sb = cons.tile([C, 6, 2], f32, tag="gb")
        ones_sb = cons.tile([1, 128], f32, tag="ones")
        eps_sb = cons.tile([C, 1], f32, tag="eps")
        nc.vector.memset(eps_sb, EPS)
        nc.sync.dma_start(out=sa1_sb, in_=sa1_d[:, :])
        nc.sync.dma_start(out=sa2_sb, in_=sa2_d[:, :])
        nc.sync.dma_start(out=cw_sb, in_=cw_d[:, :])
        nc.sync.dma_start(out=cb_sb, in_=cb_d[:, :])
        nc.sync.dma_start(out=gb_sb, in_=gb_d[:, :, :].transpose([2, 0, 1]))
        onesr_sb = cons.tile([1, 128], f32r, tag="onesr")
        nc.vector.memset(ones_sb, 1.0)
        nc.vector.tensor_copy(out=onesr_sb, in_=ones_sb)

        def load_weights(dram, nsl, cols):
            wt = wts.tile([C, nsl, cols], f32r, tag="w")
            nc.sync.dma_start(out=wt, in_=dram[:, :, :])
            return wt

        # ---------------- generic helpers ----------------
        def all_reduce(sb_in, k, groups):
            din = drp.tile([C, k], f32, tag="arin")
            dout = drp.tile([C, k], f32, tag="arout")
            nc.sync.dma_start(out=din, in_=sb_in)
            nc.gpsimd.collective_compute(
                "AllReduce", AL.add, replica_groups=groups,
                ins=[din[:, :].opt()], outs=[dout[:, :].opt()])
            sb_out = smc.tile([C, k], f32, tag=f"ar{k}_{len(_ar_cnt)}")
            _ar_cnt.append(0)
            nc.sync.dma_start(out=sb_out, in_=dout)
            return sb_out

        _ar_cnt = []

        def sums_from_mv(mv, count, p=C):
            """mv [p,2] (mean, biased var) -> packed [p,2] (sum, sum_sq)."""
            pk = smc.tile([p, 2], f32, tag=f"pk{len(_pk_cnt)}")
            _pk_cnt.append(0)
            nc.vector.tensor_scalar_mul(out=pk[:, 0:1], in0=mv[:, 0:1],
                                        scalar1=float(count))
            # e2 = (var + mean^2) * count
            nc.vector.tensor_mul(out=pk[:, 1:2], in0=mv[:, 0:1], in1=mv[:, 0:1])
            nc.vector.tensor_add(out=pk[:, 1:2], in0=pk[:, 1:2], in1=mv[:, 1:2])
            nc.vector.tensor_scalar_mul(out=pk[:, 1:2], in0=pk[:, 1:2],
                                        scalar1=float(count))
            return pk

        def mu_rstd_from_sums(gl, total, p=C):
            """gl [p,2] global (sum, sumsq) -> (mu [p,1], rstd [p,1])."""
            n = len(_mr_cnt)
            _mr_cnt.append(0)
            mu = smc.tile([p, 1], f32, tag=f"mu{n}")
            rs = smc.tile([p, 1], f32, tag=f"rs{n}")
            tv = smc.tile([p, 1], f32, tag=f"tv{n}")
            nc.vector.tensor_scalar_mul(out=mu, in0=gl[:, 0:1],
                                        scalar1=1.0 / total)
            nc.vector.tensor_scalar_mul(out=tv, in0=gl[:, 1:2],
                                        scalar1=1.0 / total)
            nc.vector.tensor_mul(out=rs, in0=mu, in1=mu)
            nc.vector.tensor_sub(out=tv, in0=tv, in1=rs)      # var
            nc.scalar.activation(out=tv, in_=tv, func=AF.Sqrt,
                                 bias=eps_sb[0:p, :], scale=1.0)
            nc.vector.reciprocal(out=rs, in_=tv)
            return mu, rs

        _pk_cnt = []
        _mr_cnt = []

        def bn_flat_stats(src_f32, p, flat_start, flat_len):
            """bn_stats over a contiguous flat span (pads must be zeroed;
            zeros only dilute mean/var, raw sums are unaffected)."""
            sf = flat(src_f32)
            nchunks = (flat_len + 511) // 512
            stats = sm.tile([p, nchunks, 6], f32, tag="st")
            for j in range(nchunks):
                a = flat_start + 512 * j
                b = min(flat_start + flat_len, a + 512)
                nc.vector.bn_stats(out=stats[:, j, :], in_=sf[0:p, a:b])
            mv = sm.tile([p, 2], f32, tag="mv")
            nc.vector.bn_aggr(out=mv, in_=stats)
            return mv

        def _build_body():
            # ================= stage 0: input build =================
            xs_sb = stage.tile([C, XROWS, W0], f32, tag="stage")
            nc.sync.dma_start(out=xs_sb, in_=xs_d[:, :, :])

            xp = stage.tile([C, 87, ST], f32, tag="stage")
            xpr = xp.bitcast(f32r)
            nc.vector.memset(xp, 0.0)
            # interior: P1 rows 2..85 = x rows 0..83 ; cols 2..129 = x cols 0..127
            nc.vector.tensor_copy(out=xpr[:, 2:86, 2:130], in_=xs_sb[:, 0:84, :])
            # P1 row 1 = x row 1
            nc.vector.tensor_copy(out=xpr[:, 1:2, 2:130], in_=xs_sb[:, 1:2, :])
            # col 1 = x col 1 ; col 130 = x col 126
            nc.vector.tensor_copy(out=xpr[:, 2:86, 1:2], in_=xs_sb[:, 0:84, 1:2])
            nc.vector.tensor_copy(out=xpr[:, 1:2, 1:2], in_=xs_sb[:, 1:2, 1:2])
            nc.vector.tensor_copy(out=xpr[:, 2:86, 130:131],
                                  in_=xs_sb[:, 0:84, 126:127])
            nc.vector.tensor_copy(out=xpr[:, 1:2, 130:131],
                                  in_=xs_sb[:, 1:2, 126:127])
            if debug_taps:
                nc.sync.dma_start(out=taps["xp"][:, :, :], in_=xp)
            ckpt(0)

            # ================= conv1 =================
            w1_sb = load_weights(w1_d, 9, C)
            out1 = stage.tile([C, 84, ST], f32, tag="stage")
            xp_f = flat(xpr)
            N1 = 414

            def conv1_tile(t):
                pt = cps.tile([C, N1], f32, tag="cps")
                ob = 3 * t * ST
                i = 0
                for di in range(3):
                    for dj in range(3):
                        nc.tensor.matmul(
                            out=pt[:, :], lhsT=w1_sb[:, di * 3 + dj, :],
                            rhs=xp_f[:, ob + di * ST + dj: ob + di * ST + dj + N1],
                            start=(i == 0), stop=(i == 8))
                        i += 1
                nc.scalar.copy(out=flat(out1)[:, ob:ob + N1], in_=pt[:, :])

            for t in range(22):
                conv1_tile(t)
            # IN1 stats: rows 0..64 (zero the 8 junk cols first, flat chunks)
            nc.vector.memset(out1[:, 0:65, 130:138], 0.0)
            mv1 = bn_flat_stats(out1, C, 0, 65 * ST)
            pk1 = sums_from_mv(mv1, 65 * ST)
            gl1 = all_reduce(pk1, 2, PAIRS)
            for t in range(22, 28):
                conv1_tile(t)
            mu1, rs1 = mu_rstd_from_sums(gl1, CNT1_TOT)
            if debug_taps:
                nc.sync.dma_start(out=taps["out1"][:, :, :], in_=out1)
            ckpt(1)
            if debug_taps:
                nc.sync.dma_start(out=taps["stats1"][:, :], in_=gl1)
            ckpt(2)

            # negated bias for ACT: relu(x*rs1 - mu1*rs1)
            nb1 = smc.tile([C, 1], f32, tag="nb1")
            nc.vector.tensor_mul(out=nb1, in0=mu1, in1=rs1)
            nc.vector.tensor_scalar_mul(out=nb1, in0=nb1, scalar1=-1.0)

            # ================= r1p build =================
            r1p = stage.tile([C, 88, ST], f32, tag="stage")
            r1r = r1p.bitcast(f32r)
            nc.vector.memset(r1p, 0.0)

            def rel(dst, src):
                nc.scalar.activation(out=dst, in_=src, func=AF.Relu,
                                     bias=nb1, scale=rs1)

            rel(r1r[:, 3:87, 2:132], out1[:, 0:84, 0:130])
            rel(r1r[:, 2:3, 2:132], out1[:, 1:2, 0:130])
            rel(r1r[:, 3:87, 1:2], out1[:, 0:84, 1:2])
            rel(r1r[:, 2:3, 1:2], out1[:, 1:2, 1:2])
            rel(r1r[:, 3:87, 132:133], out1[:, 0:84, 128:129])
            rel(r1r[:, 2:3, 132:133], out1[:, 1:2, 128:129])
            if debug_taps:
                nc.sync.dma_start(out=taps["r1p"][:, :, :], in_=r1p)
            ckpt(3)

            # ================= conv2 =================
            w2_sb = load_weights(w2_d, 9, C)
            out2 = stage.tile([C, 84, ST], f32, tag="stage")
            r1_f = flat(r1r)

            def conv2_tile(t):
                pt = cps.tile([C, N1], f32, tag="cps")
                ob = 3 * t * ST
                i = 0
                for di in range(3):
                    for dj in range(3):
                        off = (di + 1) * ST + (dj - 3)
                        nc.tensor.matmul(
                            out=pt[:, :], lhsT=w2_sb[:, di * 3 + dj, :],
                            rhs=r1_f[:, ob + off: ob + off + N1],
                            start=(i == 0), stop=(i == 8))
                        i += 1
                nc.scalar.copy(out=flat(out2)[:, ob:ob + N1], in_=pt[:, :])

            for t in range(22):
                conv2_tile(t)
            if stage_limit == 31:
                for t in range(22, 28):
                    conv2_tile(t)
                if debug_taps:
                    nc.sync.dma_start(out=taps["out2"][:, :, :], in_=out2)
            if stage_limit == 31:
                raise _StopBuild()
            # IN2 stats + dct-mean dot:  band rows 0..65, cols at slots 3..134
            nc.vector.memset(out2[:, 0:66, 0:3], 0.0)
            nc.vector.memset(out2[:, 0:66, 135:138], 0.0)
            mv2 = bn_flat_stats(out2, C, 0, 66 * ST)
            if stage_limit == 32:
                raise _StopBuild()
            acc = sm.tile([C, 22], f32, tag="dotacc")
            for j in range(22):
                wvt = wvp.tile([C, 3, H2], f32, tag="wv")
                nc.sync.dma_start(
                    out=wvt, in_=wv_d[3 * j:3 * j + 3, :].partition_broadcast(C))
                scr = wvp.tile([C, 3, H2], f32, tag="scr")
                nc.vector.tensor_mul(out=scr,
                                     in0=out2[:, 3 * j:3 * j + 3, 3:135],
                                     in1=wvt)
                nc.vector.tensor_reduce(out=acc[:, j:j + 1], in_=scr,
                                        axis=AX.XY, op=AL.add)
            dotw = smc.tile([C, 1], f32, tag="dotw")
            nc.vector.tensor_reduce(out=dotw, in_=acc, axis=AX.X, op=AL.add)
            if stage_limit == 33:
                raise _StopBuild()
            pk2 = sums_from_mv(mv2, 66 * ST)
            pk2b = smc.tile([C, 3], f32, tag="pk2b")
            nc.vector.tensor_copy(out=pk2b[:, 0:2], in_=pk2)
            nc.vector.tensor_copy(out=pk2b[:, 2:3], in_=dotw)
            gl2 = all_reduce(pk2b, 3, PAIRS)
            if stage_limit == 34:
                raise _StopBuild()
            for t in range(22, 28):
                conv2_tile(t)
            mu2, rs2 = mu_rstd_from_sums(gl2, CNT2_TOT)
            if debug_taps:
                nc.sync.dma_start(out=taps["out2"][:, :, :], in_=out2)
            ckpt(4)
            if debug_taps:
                nc.sync.dma_start(out=taps["stats2"][:, :], in_=gl2)

            # ================= SFOM gate =================
            # m = rs2 * (dotw_glob - mu2 * S_w)
            m_sb = smc.tile([C, 1], f32, tag="m")
            nc.vector.tensor_scalar_mul(out=m_sb, in0=mu2, scalar1=-S_W)
            nc.vector.tensor_add(out=m_sb, in0=m_sb, in1=gl2[:, 2:3])
            nc.vector.tensor_mul(out=m_sb, in0=m_sb, in1=rs2)
            # gate = sigmoid(relu(m @ sa1) @ sa2)
            p_r = mps.tile([8, 1], f32, tag="mps")
            nc.tensor.matmul(out=p_r, lhsT=sa1_sb, rhs=m_sb, start=True, stop=True)
            relu_sb = smc.tile([8, 1], f32, tag="relu8")
            nc.scalar.activation(out=relu_sb, in_=p_r, func=AF.Relu,
                                 bias=0.0, scale=1.0)
            p_g = mps.tile([C, 1], f32, tag="mps")
            nc.tensor.matmul(out=p_g, lhsT=sa2_sb, rhs=relu_sb,
                             start=True, stop=True)
            gate = smc.tile([C, 1], f32, tag="gate")
            nc.scalar.activation(out=gate, in_=p_g, func=AF.Sigmoid,
                                 bias=0.0, scale=1.0)
            if debug_taps:
                nc.sync.dma_start(out=taps["gate"][:, :], in_=gate)
            ckpt(5)
            # s_sig = rs2 * (1+gate)/2
            ssig = smc.tile([C, 1], f32, tag="ssig")
            nc.vector.tensor_scalar(out=ssig, in0=gate, scalar1=0.5, scalar2=0.5,
                                    op0=AL.mult, op1=AL.add)
            nc.vector.tensor_mul(out=ssig, in0=ssig, in1=rs2)
            nbs = smc.tile([C, 1], f32, tag="nbs")     # -mu2*ssig
            nc.vector.tensor_mul(out=nbs, in0=mu2, in1=ssig)
            nc.vector.tensor_scalar_mul(out=nbs, in0=nbs, scalar1=-1.0)

            # ================= SFOM apply =================
            # o2 = (out2-mu2)*rs2 ; f = sigmoid(o2*g2')*o2  (slots +4 rows in f)
            o2 = stage.tile([C, 84, ST], f32, tag="stage")
            nc.vector.tensor_scalar(out=o2[:, :, 3:135], in0=out2[:, :, 3:135],
                                    scalar1=mu2, scalar2=rs2,
                                    op0=AL.subtract, op1=AL.mult)
            ftile = fpool.tile([C, R_F + 5, ST], f32, tag="f")
            fr = ftile.bitcast(f32r)
            nc.vector.memset(ftile, 0.0)
            # sig = Sigmoid(out2*ssig + nbs)  (== sigmoid(o2*g2'))
            nc.scalar.activation(out=fr[:, 4:88, 3:135], in_=out2[:, :, 3:135],
                                 func=AF.Sigmoid, bias=nbs, scale=ssig)
            nc.vector.tensor_mul(out=fr[:, 4:88, 3:135],
                                 in0=ftile[:, 4:88, 3:135], in1=o2[:, :, 3:135])
            if debug_taps:
                nc.sync.dma_start(out=taps["f"][:, :, :], in_=ftile)
            ckpt(6)

            # ================= SPEM layers =================
            # per layer: (cin_packed_src, R, co, mm plan, psum N, combine)
            def spem_layer(lidx, src_r, wtile, co, R, NP, mms, combine, ncopies,
                           copy_cp):
                """Emit one SPEM conv layer.

                mms: list of (slice_idx, beta) matmul descriptors (lhsT slice of
                     wtile, rhs offset); combine(pt, dst_flat, ob) drains psum.
                ncopies/copy_cp: shifted-copy count and partition width for
                     K-packing of the NEXT layer's input.
                """
                S = R + 5
                h = stage.tile([128 if (ncopies or co > 64) else co, S, ST],
                               f32, tag="stage")
                hr = h.bitcast(f32r)
                nc.vector.memset(h, 0.0)
                hf = (flat(hr), flat(h))
                src_f = flat(src_r)
                ntiles = R // 3

                def conv_tile(t):
                    pt = cps.tile([mms_part, NP], f32, tag="cps")
                    ob = (4 + 3 * t) * ST
                    for i, (sl, beta) in enumerate(mms):
                        nc.tensor.matmul(
                            out=pt[:, :], lhsT=wtile[:, sl, :],
                            rhs=src_f[:, ob + beta: ob + beta + NP],
                            start=(i == 0), stop=(i == len(mms) - 1))
                    combine(pt, hf, ob)

                mms_part = 128
                for t in range(22):
                    conv_tile(t)
                nc.vector.memset(h[0:co, 4:70, 0:3], 0.0)
                nc.vector.memset(h[0:co, 4:70, 135:138], 0.0)
                mvb = bn_flat_stats(h, co, 4 * ST, 66 * ST)
                pkb = sums_from_mv(mvb, 66 * ST, p=co)
                pkb128 = smc.tile([C, 2], f32, tag=f"pkb128_{lidx}")
                if co < C:
                    nc.vector.memset(pkb128, 0.0)
                nc.vector.tensor_copy(out=pkb128[0:co, :], in_=pkb)
                glb = all_reduce(pkb128, 2, ALL8)
                for t in range(22, ntiles):
                    conv_tile(t)
                mub, rsb = mu_rstd_from_sums(glb[0:co, :], CNTB_TOT, p=co)
                # scale = gamma*rstd ; bias = beta - mu*scale
                sc = smc.tile([co, 1], f32, tag=f"sc{lidx}")
                bi = smc.tile([co, 1], f32, tag=f"bi{lidx}")
                nc.vector.tensor_mul(out=sc, in0=gb_sb[0:co, lidx, 0:1], in1=rsb)
                nc.vector.tensor_mul(out=bi, in0=mub, in1=sc)
                nc.vector.tensor_sub(out=bi, in0=gb_sb[0:co, lidx, 1:2], in1=bi)
                nc.scalar.activation(out=hr[0:co, 4:4 + R, 3:135],
                                     in_=h[0:co, 4:4 + R, 3:135],
                                     func=AF.Relu, bias=bi, scale=sc)
                if R > 66:
                    nc.vector.memset(h[0:co, 70:4 + R, 0:3], 0.0)
                    nc.vector.memset(h[0:co, 70:4 + R, 135:138], 0.0)
                for g in range(1, ncopies + 1):
                    nc.vector.tensor_copy(
                        out=hr[g * copy_cp:(g + 1) * copy_cp, :, 0:ST - g],
                        in_=hr[0:copy_cp, :, g:ST])
                if debug_taps:
                    tp = taps[f"h{lidx+1}"]
                    nc.sync.dma_start(out=tp[:, :, :], in_=h[0:tp.shape[0], :, :])
                return hr

            def drain_act(pt, hf, ob):
                nc.scalar.copy(out=hf[0][:, ob:ob + 414], in_=pt[:, 0:414])

            def mk_combine(groups, cp):
                """groups: list of (psum partition group idx, col shift).
                DVE reads at most one PSUM operand: copy then accumulate."""
                def comb(pt, hf, ob):
                    hfr, hf32 = hf
                    g0, s0 = groups[0]
                    nc.vector.tensor_copy(
                        out=hfr[0:cp, ob:ob + 414],
                        in_=pt[g0 * cp:(g0 + 1) * cp, s0:s0 + 414])
                    for g, s in groups[1:]:
                        nc.vector.tensor_add(
                            out=hfr[0:cp, ob:ob + 414],
                            in0=hf32[0:cp, ob:ob + 414],
                            in1=pt[g * cp:(g + 1) * cp, s:s + 414])
                return comb

            # L1: 128->32, Mpack4: psum[g*32+co] <-> out[n-g]
            s1_sb = load_weights(s_d[0], 14, 128)
            mms1 = [(di * 2 + s, (di - 3) * ST + 4 * s - 3)
                    for di in range(7) for s in range(2)]
            h1 = spem_layer(0, fr, s1_sb, 32, R_H[0], 418, mms1,
                            mk_combine([(0, 0), (1, 1), (2, 2), (3, 3)], 32),
                            3, 32)
            # L2: 32->64, Kpack4 + Mpack2(supergroups +4): psum[G*64+co]<->out[n-4G]
            ckpt(7)
            s2_sb = load_weights(s_d[1], 7, 128)
            mms2 = [(di, (di - 3) * ST - 3) for di in range(7)]
            h2 = spem_layer(1, h1, s2_sb, 64, R_H[1], 418, mms2,
                            mk_combine([(0, 0), (1, 4)], 64), 1, 64)
            # L3: 64->128, Kpack2: 4 dj-groups
            ckpt(8)
            s3_sb = load_weights(s_d[2], 28, 128)
            mms3 = [(di * 4 + g, (di - 3) * ST + 2 * g - 3)
                    for di in range(7) for g in range(4)]
            h3 = spem_layer(2, h2, s3_sb, 128, R_H[2], 414, mms3, drain_act, 0, 0)
            # L4: 128->128 plain
            ckpt(9)
            s4_sb = load_weights(s_d[3], 49, 128)
            mms4 = [(di * 7 + dj, (di - 3) * ST + dj - 3)
                    for di in range(7) for dj in range(7)]
            h4 = spem_layer(3, h3, s4_sb, 128, R_H[3], 414, mms4, drain_act, 0, 0)
            # L5: 128->64, Mpack2: psum[g*64+co] <-> out[n-g]
            ckpt(10)
            s5_sb = load_weights(s_d[4], 28, 128)
            mms5 = [(di * 4 + st, (di - 3) * ST + 2 * st - 3)
                    for di in range(7) for st in range(4)]
            h5 = spem_layer(4, h4, s5_sb, 64, R_H[4], 416, mms5,
                            mk_combine([(0, 0), (1, 1)], 64), 1, 64)
            # L6: 64->32, Kpack2 + Mpack2: psum[G*32+co] <-> out[n-2G]
            ckpt(11)
            s6_sb = load_weights(s_d[5], 14, 128)
            mms6 = [(di * 2 + T, (di - 3) * ST + 4 * T - 3)
                    for di in range(7) for T in range(2)]
            h6 = spem_layer(5, h5, s6_sb, 32, R_H[5], 416, mms6,
                            mk_combine([(0, 0), (1, 2)], 32), 0, 0)

            ckpt(12)
            # ================= 1x1 conv + finale =================
            h6f32 = flat(h6.bitcast(f32))
            f_f = flat(ftile)
            for t in range(22):
                ob = (4 + 3 * t) * ST
                p7 = mps.tile([1, 414], f32, tag="mps")
                nc.tensor.matmul(out=p7, lhsT=cw_sb,
                                 rhs=h6f32[:, ob:ob + 414], start=True, stop=True)
                sa_c = sap.tile([1, 414], f32r, tag="sa")
                nc.scalar.activation(out=sa_c, in_=p7,
                                     func=AF.Sigmoid, bias=cb_sb[0:1, 0:1],
                                     scale=1.0)
                prep = mps.tile([128, 414], f32, tag="mps")
                nc.tensor.matmul(out=prep, lhsT=onesr_sb,
                                 rhs=sa_c, start=True, stop=True)
                ot = otp.tile([C, 414], f32, tag="ot")
                nc.vector.tensor_mul(out=ot, in0=prep, in1=f_f[:, ob:ob + 414])
                otv = ot.rearrange("p (r c) -> p r c", c=ST)
                nc.sync.dma_start(out=out_d[:, 3 * t:3 * t + 3, :],
                                  in_=otv[:, :, 3:135])


        try:
            _build_body()
        except _StopBuild:
            pass
        for p in [drp, mps, cps, sap, otp, wvp, smc, sm, cons, wts, fpool,
                  stage]:
            p.release()

    nc.compile()
    _PROGRAM_CACHE[key] = (nc, taps)
    return nc, taps


# ----------------------------------------------------------------------------
# host-side packing
# ----------------------------------------------------------------------------

def _pack_core_inputs(inputs, core):
    b, half = core // 2, core % 2
    flip = (half == 1)

    def fd(w):          # flip di (axis 2) of [co, ci, kh, kw]
        return w[:, :, ::-1, :] if flip else w

    x = inputs['x'][b]
    if flip:
        x = x[:, ::-1, :]
    xs = np.ascontiguousarray(x[:, 0:XROWS, :], np.float32)

    w1 = fd(inputs['conv1_w'])
    w2 = fd(inputs['conv2_w'])
    w1p = np.ascontiguousarray(
        np.transpose(w1, (2, 3, 1, 0)).reshape(9, C, C).transpose(1, 0, 2),
        np.float32)
    w2p = np.ascontiguousarray(
        np.transpose(w2, (2, 3, 1, 0)).reshape(9, C, C).transpose(1, 0, 2),
        np.float32)

    dws = [fd(inputs[f'dw{i}']) for i in range(1, 7)]

    def lhsT(w, di, dj):
        return w[:, :, di, dj].T        # [ci, co]

    # s1 [C, 14, 128]: idx di*2+s ; cols g*32+co = dj=4s+g
    s1 = np.zeros((C, 14, 128), np.float32)
    for di in range(7):
        for s in range(2):
            for g in range(4):
                dj = 4 * s + g
                if dj < 7:
                    s1[:, di * 2 + s, 32 * g:32 * g + 32] = lhsT(dws[0], di, dj)
    # s2 [C, 7, 128]: idx di ; rows s*32+ci ; cols G*64+co = dj=4G+s
    s2 = np.zeros((C, 7, 128), np.float32)
    for di in range(7):
        for s in range(4):
            for G in range(2):
                dj = 4 * G + s
                if dj < 7:
                    s2[32 * s:32 * s + 32, di, 64 * G:64 * G + 64] = \
                        lhsT(dws[1], di, dj)
    # s3 [C, 28, 128]: idx di*4+grp ; rows s*64+ci ; dj=2grp+s
    s3 = np.zeros((C, 28, 128), np.float32)
    for di in range(7):
        for grp in range(4):
            for s in range(2):
                dj = 2 * grp + s
                if dj < 7:
                    s3[64 * s:64 * s + 64, di * 4 + grp, :] = \
                        lhsT(dws[2], di, dj)
    # s4 [C, 49, 128]
    s4 = np.zeros((C, 49, 128), np.float32)
    for di in range(7):
        for dj in range(7):
            s4[:, di * 7 + dj, :] = lhsT(dws[3], di, dj)
    # s5 [C, 28, 128]: idx di*4+st ; cols g*64+co = dj=2st+g
    s5 = np.zeros((C, 28, 128), np.float32)
    for di in range(7):
        for st in range(4):
            for g in range(2):
                dj = 2 * st + g
                if dj < 7:
                    s5[:, di * 4 + st, 64 * g:64 * g + 64] = \
                        lhsT(dws[4], di, dj)
    # s6 [C, 14, 128]: idx di*2+T ; rows s*64+ci ; cols G*32+co = dj=4T+2G+s
    # (cols 64..127 stay zero: M padded to 128 for the f32r col_grp rule)
    s6 = np.zeros((C, 14, 128), np.float32)
    for di in range(7):
        for T in range(2):
            for G in range(2):
                for s in range(2):
                    dj = 4 * T + 2 * G + s
                    if dj < 7:
                        s6[64 * s:64 * s + 64, di * 2 + T,
                           32 * G:32 * G + 32] = lhsT(dws[5], di, dj)

    wvec = dct_mean_weights().reshape(H2, H2)
    if flip:
        wv = np.ascontiguousarray(wvec[::-1, :][0:HB], np.float32)
    else:
        wv = np.ascontiguousarray(wvec[0:HB], np.float32)

    gb = np.zeros((6, 2, C), np.float32)
    for k in range(6):
        g = inputs[f'bg{k+1}']
        bb = inputs[f'bb{k+1}']
        gb[k, 0, :len(g)] = g
        gb[k, 1, :len(bb)] = bb

    return {
        'xs': xs, 'w1': w1p, 'w2c': w2p,
        's1': s1, 's2': s2, 's3': s3, 's4': s4, 's5': s5, 's6': s6,
        'cw': np.ascontiguousarray(inputs['spem_cw'][0, :, 0, 0][:, None],
                                   np.float32),
        'cb': np.asarray(inputs['spem_cb'], np.float32).reshape(1, 1),
        'sa1t': np.ascontiguousarray(inputs['sa_w1'].T, np.float32),
        'sa2t': np.ascontiguousarray(inputs['sa_w2'].T, np.float32),
        'gb': gb, 'wv': wv,
    }


def run_cores(inputs, trace=False, debug_taps=False, stage_limit=99):
    _install_ntff_hook()
    from concourse.bass_utils import run_bass_kernel_spmd
    nc, taps = build_program(debug_taps=debug_taps, stage_limit=stage_limit)
    in_maps = [_pack_core_inputs(inputs, c) for c in range(NCORES)]
    res = run_bass_kernel_spmd(nc, in_maps, list(range(NCORES)), trace=trace)
    return res


def kernel(**inputs):
    res = run_cores(inputs)
    full = np.empty((B, C, H2, H2), np.float32)
    for b in range(B):
        full[b, :, 0:HB, :] = res.results[2 * b]["out"]
        full[b, :, HB:H2, :] = res.results[2 * b + 1]["out"][:, ::-1, :]
    return full



# revision 19
# speedup vs baseline: 1.0320x; 1.0320x over previous
"""Trainium2 Bass kernel for nn_CBAM_84799834292534.

Strategy:
- 8 cores = 4 batch samples x 2 vertical halves. Half-1 cores receive
  row-flipped inputs/weights so every core runs the identical program
  ("local top" = its outer image edge, halo rows toward the cut edge).
- Halos handled by redundant compute (no halo exchange).
- SFOM's DCT gating collapses analytically: idct(gate*dct(x)) == gate*x,
  and mean(dct(x)) == dot(x, w) with w = idct_ortho(ones)/N.
- Convs are shifted matmuls with channels on partitions, f32r dtype.
  K-packing (stacking dj-shifts along the contraction dim via shifted
  input copies) and M-packing (stacking dj-shifts along output channels,
  combined with shifted PSUM adds) keep the PE near full utilization.
- InstanceNorm/BatchNorm/DCT-mean stats use partial sums + tiny
  AllReduces (pair groups for per-sample stats, all-8 for BatchNorm).
  Stat bands are computed first so the AllReduce overlaps halo tiles.
"""
import sys
import types

sys.path.insert(0, '/opt/trn_rl_repo')
import numpy as np
import ml_dtypes

BF16 = ml_dtypes.bfloat16

B, C, H0, W0 = 4, 128, 128, 128
H1 = 130          # after conv1
H2 = 132          # after conv2 (final spatial)
ST = 138          # uniform row stride of on-chip layouts
EPS = 1e-5
NCORES = 8
HB = 66           # output band rows per core

XROWS = 84        # x rows needed per core
R_F = 84          # f (SFOM output) data rows
R_H = [81, 78, 75, 72, 69, 66]   # SPEM layer output rows (local)

CNT1_LOC = 65 * 130
CNT1_TOT = 130 * 130
CNT2_LOC = 66 * 132
CNT2_TOT = 132 * 132
CNTB_LOC = 66 * 132
CNTB_TOT = 8 * 66 * 132


def _idct_ortho_np(Xin):
    """numpy copy of the reference _idct_ortho (float64)."""
    X = np.asarray(Xin, np.float64)
    N = X.shape[-1]
    scale = np.full(N, np.sqrt(N / 2.0))
    scale[0] = np.sqrt(float(N))
    Xv = X * scale
    k = np.arange(N) * (np.pi / (2.0 * N))
    Wr, Wi = np.cos(k), np.sin(k)
    Vti = np.concatenate([np.zeros(1), -Xv[::-1][:-1]])
    V = (Xv * Wr - Vti * Wi) + 1j * (Xv * Wi + Vti * Wr)
    v = np.fft.ifft(V).real
    out = np.zeros_like(v)
    out[0::2] = v[: (N + 1) // 2]
    out[1::2] = v[::-1][: N // 2]
    return out


def dct_mean_weights():
    """w such that mean(dct_ortho(x)) == dot(x, w), x of length H2*W2."""
    N = H2 * H2
    return _idct_ortho_np(np.ones(N)) / N


def _install_ntff_hook():
    if "antenv.axon_hooks" in sys.modules:
        return
    mod = types.ModuleType("antenv.axon_hooks")
    _state = {"hook": None}
    mod.set_axon_ntff_profile_hook = lambda h: _state.__setitem__("hook", h)
    mod.get_axon_ntff_profile_hook = lambda: _state["hook"]
    sys.modules["antenv.axon_hooks"] = mod
    try:
        from trn_agent_boot.trn_boot import _ntff_profile_via_ctypes
        mod.set_axon_ntff_profile_hook(
            _ntff_profile_via_ctypes('/opt/axon/libaxon_pjrt.so'))
    except Exception:
        pass


# ----------------------------------------------------------------------------
# program build
# ----------------------------------------------------------------------------

_PROGRAM_CACHE = {}


class _StopBuild(Exception):
    pass


def build_program(debug_taps=False, stage_limit=99):
    key = (bool(debug_taps), stage_limit)
    if key in _PROGRAM_CACHE:
        return _PROGRAM_CACHE[key]

    import concourse.bacc as bacc
    import concourse.tile as tile
    from concourse import mybir

    f32 = mybir.dt.float32
    f32r = mybir.dt.float32r
    bf16 = mybir.dt.bfloat16
    AF = mybir.ActivationFunctionType
    AL = mybir.AluOpType
    AX = mybir.AxisListType

    nc = bacc.Bacc("TRN2", target_bir_lowering=False)

    # ---------------- external tensors ----------------
    xs_d = nc.dram_tensor("xs", [C, XROWS, W0], bf16, kind="ExternalInput")
    w1_d = nc.dram_tensor("w1", [C, 9, C], bf16, kind="ExternalInput")
    w2_d = nc.dram_tensor("w2c", [C, 9, C], bf16, kind="ExternalInput")
    s_d = [
        nc.dram_tensor("s1", [C, 14, 128], bf16, kind="ExternalInput"),
        nc.dram_tensor("s2", [C, 7, 128], bf16, kind="ExternalInput"),
        nc.dram_tensor("s3", [C, 28, 128], bf16, kind="ExternalInput"),
        nc.dram_tensor("s4", [C, 49, 128], bf16, kind="ExternalInput"),
        nc.dram_tensor("s5", [C, 28, 128], bf16, kind="ExternalInput"),
        nc.dram_tensor("s6", [C, 14, 128], bf16, kind="ExternalInput"),
    ]
    cw_d = nc.dram_tensor("cw", [32, 1], bf16, kind="ExternalInput")
    cb_d = nc.dram_tensor("cb", [1, 1], f32, kind="ExternalInput")
    sa1_d = nc.dram_tensor("sa1t", [C, 8], f32, kind="ExternalInput")
    sa2_d = nc.dram_tensor("sa2t", [8, C], f32, kind="ExternalInput")
    gb_d = nc.dram_tensor("gb", [6, 2, C], f32, kind="ExternalInput")
    wv_d = nc.dram_tensor("wv", [HB, H2], bf16, kind="ExternalInput")
    out_d = nc.dram_tensor("out", [C, HB, H2], f32, kind="ExternalOutput")
    S_W = float(dct_mean_weights().sum())

    taps = {}
    if debug_taps:
        def tap(name, shape, dt=bf16):
            taps[name] = nc.dram_tensor("tap_" + name, shape, dt,
                                        kind="ExternalOutput")
        tap("xp", [C, 87, ST])
        tap("out1", [C, 84, ST])
        tap("r1p", [C, 88, ST])
        tap("out2", [C, 84, ST])
        tap("f", [C, 89, ST])
        tap("stats1", [C, 2], f32)
        tap("stats2", [C, 3], f32)
        tap("gate", [C, 1], f32)
        for k in range(6):
            tap(f"h{k+1}", [128 if k < 5 else 32, R_H[k] + 5, ST])

    PAIRS = [[0, 1], [2, 3], [4, 5], [6, 7]]
    ALL8 = [list(range(NCORES))]

    with tile.TileContext(nc) as tc:
        stage = tc.alloc_tile_pool(name="stage", bufs=2)
        fpool = tc.alloc_tile_pool(name="fpool", bufs=1)
        wts = tc.alloc_tile_pool(name="wts", bufs=1)
        cons = tc.alloc_tile_pool(name="cons", bufs=1)
        sm = tc.alloc_tile_pool(name="sm", bufs=2)
        smc = tc.alloc_tile_pool(name="smc", bufs=1)
        wvp = tc.alloc_tile_pool(name="wvp", bufs=2)
        otp = tc.alloc_tile_pool(name="otp", bufs=2)
        sap = tc.alloc_tile_pool(name="sap", bufs=3)
        cps = tc.alloc_tile_pool(name="cps", bufs=6, space="PSUM")
        mps = tc.alloc_tile_pool(name="mps", bufs=2, space="PSUM")
        drp = tc.alloc_tile_pool(name="drp", bufs=1, space="DRAM")

        def flat(t):
            return t.rearrange("p r c -> p (r c)")

        def ckpt(n):
            if stage_limit <= n:
                raise _StopBuild()

        # ---------------- constants ----------------
        sa1_sb = cons.tile([C, 8], f32, tag="sa1")
        sa2_sb = cons.tile([8, C], f32, tag="sa2")
        cw_sb = cons.tile([32, 1], bf16, tag="cw")
        cb_sb = cons.tile([1, 1], f32, tag="cb")
        gb_sb = cons.tile([C, 6, 2], f32, tag="gb")
        ones_sb = cons.tile([1, 128], f32, tag="ones")
        eps_sb = cons.tile([C, 1], f32, tag="eps")
        nc.vector.memset(eps_sb, EPS)
        nc.sync.dma_start(out=sa1_sb, in_=sa1_d[:, :])
        nc.sync.dma_start(out=sa2_sb, in_=sa2_d[:, :])
        nc.sync.dma_start(out=cw_sb, in_=cw_d[:, :])
        nc.sync.dma_start(out=cb_sb, in_=cb_d[:, :])
        nc.sync.dma_start(out=gb_sb, in_=gb_d[:, :, :].transpose([2, 0, 1]))
        onesr_sb = cons.tile([1, 128], bf16, tag="onesr")
        nc.vector.memset(ones_sb, 1.0)
        nc.vector.tensor_copy(out=onesr_sb, in_=ones_sb)

        def load_weights(dram, nsl, cols):
            wt = wts.tile([C, nsl, cols], bf16, tag="w")
            nc.sync.dma_start(out=wt, in_=dram[:, :, :])
            return wt

        # ---------------- generic helpers ----------------
        def all_reduce(sb_in, k, groups):
            din = drp.tile([C, k], f32, tag="arin")
            dout = drp.tile([C, k], f32, tag="arout")
            nc.sync.dma_start(out=din, in_=sb_in)
            nc.gpsimd.collective_compute(
                "AllReduce", AL.add, replica_groups=groups,
                ins=[din[:, :].opt()], outs=[dout[:, :].opt()])
            sb_out = smc.tile([C, k], f32, tag=f"ar{k}_{len(_ar_cnt)}")
            _ar_cnt.append(0)
            nc.sync.dma_start(out=sb_out, in_=dout)
            return sb_out

        _ar_cnt = []

        def sums_from_mv(mv, count, p=C):
            """mv [p,2] (mean, biased var) -> packed [p,2] (sum, sum_sq)."""
            pk = smc.tile([p, 2], f32, tag=f"pk{len(_pk_cnt)}")
            _pk_cnt.append(0)
            nc.vector.tensor_scalar_mul(out=pk[:, 0:1], in0=mv[:, 0:1],
                                        scalar1=float(count))
            # e2 = (var + mean^2) * count
            nc.vector.tensor_mul(out=pk[:, 1:2], in0=mv[:, 0:1], in1=mv[:, 0:1])
            nc.vector.tensor_add(out=pk[:, 1:2], in0=pk[:, 1:2], in1=mv[:, 1:2])
            nc.vector.tensor_scalar_mul(out=pk[:, 1:2], in0=pk[:, 1:2],
                                        scalar1=float(count))
            return pk

        def mu_rstd_from_sums(gl, total, p=C):
            """gl [p,2] global (sum, sumsq) -> (mu [p,1], rstd [p,1])."""
            n = len(_mr_cnt)
            _mr_cnt.append(0)
            mu = smc.tile([p, 1], f32, tag=f"mu{n}")
            rs = smc.tile([p, 1], f32, tag=f"rs{n}")
            tv = smc.tile([p, 1], f32, tag=f"tv{n}")
            nc.vector.tensor_scalar_mul(out=mu, in0=gl[:, 0:1],
                                        scalar1=1.0 / total)
            nc.vector.tensor_scalar_mul(out=tv, in0=gl[:, 1:2],
                                        scalar1=1.0 / total)
            nc.vector.tensor_mul(out=rs, in0=mu, in1=mu)
            nc.vector.tensor_sub(out=tv, in0=tv, in1=rs)      # var
            nc.scalar.activation(out=tv, in_=tv, func=AF.Sqrt,
                                 bias=eps_sb[0:p, :], scale=1.0)
            nc.vector.reciprocal(out=rs, in_=tv)
            return mu, rs

        _pk_cnt = []
        _mr_cnt = []

        def bn_flat_stats(src_f32, p, flat_start, flat_len):
            """bn_stats over a contiguous flat span (pads must be zeroed;
            zeros only dilute mean/var, raw sums are unaffected)."""
            sf = flat(src_f32)
            nchunks = (flat_len + 511) // 512
            stats = sm.tile([p, nchunks, 6], f32, tag="st")
            for j in range(nchunks):
                a = flat_start + 512 * j
                b = min(flat_start + flat_len, a + 512)
                nc.vector.bn_stats(out=stats[:, j, :], in_=sf[0:p, a:b])
            mv = sm.tile([p, 2], f32, tag="mv")
            nc.vector.bn_aggr(out=mv, in_=stats)
            return mv

        def _build_body():
            # ================= stage 0: input build =================
            xs_sb = stage.tile([C, XROWS, W0], bf16, tag="stage")
            nc.sync.dma_start(out=xs_sb, in_=xs_d[:, :, :])

            xp = stage.tile([C, 87, ST], bf16, tag="stage")
            xpr = xp
            nc.vector.memset(xp, 0.0)
            # interior: P1 rows 2..85 = x rows 0..83 ; cols 2..129 = x cols 0..127
            nc.vector.tensor_copy(out=xpr[:, 2:86, 2:130], in_=xs_sb[:, 0:84, :])
            # P1 row 1 = x row 1
            nc.vector.tensor_copy(out=xpr[:, 1:2, 2:130], in_=xs_sb[:, 1:2, :])
            # col 1 = x col 1 ; col 130 = x col 126
            nc.vector.tensor_copy(out=xpr[:, 2:86, 1:2], in_=xs_sb[:, 0:84, 1:2])
            nc.vector.tensor_copy(out=xpr[:, 1:2, 1:2], in_=xs_sb[:, 1:2, 1:2])
            nc.vector.tensor_copy(out=xpr[:, 2:86, 130:131],
                                  in_=xs_sb[:, 0:84, 126:127])
            nc.vector.tensor_copy(out=xpr[:, 1:2, 130:131],
                                  in_=xs_sb[:, 1:2, 126:127])
            if debug_taps:
                nc.sync.dma_start(out=taps["xp"][:, :, :], in_=xp)
            ckpt(0)

            # ================= conv1 =================
            w1_sb = load_weights(w1_d, 9, C)
            out1 = stage.tile([C, 84, ST], bf16, tag="stage")
            xp_f = flat(xpr)
            N1 = 414

            def conv1_tile(t):
                pt = cps.tile([C, N1], f32, tag="cps")
                ob = 3 * t * ST
                i = 0
                for di in range(3):
                    for dj in range(3):
                        nc.tensor.matmul(
                            out=pt[:, :], lhsT=w1_sb[:, di * 3 + dj, :],
                            rhs=xp_f[:, ob + di * ST + dj: ob + di * ST + dj + N1],
                            start=(i == 0), stop=(i == 8))
                        i += 1
                nc.scalar.copy(out=flat(out1)[:, ob:ob + N1], in_=pt[:, :])

            for t in range(22):
                conv1_tile(t)
            # IN1 stats: rows 0..64 (zero the 8 junk cols first, flat chunks)
            nc.vector.memset(out1[:, 0:65, 130:138], 0.0)
            mv1 = bn_flat_stats(out1, C, 0, 65 * ST)
            pk1 = sums_from_mv(mv1, 65 * ST)
            gl1 = all_reduce(pk1, 2, PAIRS)
            for t in range(22, 28):
                conv1_tile(t)
            mu1, rs1 = mu_rstd_from_sums(gl1, CNT1_TOT)
            if debug_taps:
                nc.sync.dma_start(out=taps["out1"][:, :, :], in_=out1)
            ckpt(1)
            if debug_taps:
                nc.sync.dma_start(out=taps["stats1"][:, :], in_=gl1)
            ckpt(2)

            # negated bias for ACT: relu(x*rs1 - mu1*rs1)
            nb1 = smc.tile([C, 1], f32, tag="nb1")
            nc.vector.tensor_mul(out=nb1, in0=mu1, in1=rs1)
            nc.vector.tensor_scalar_mul(out=nb1, in0=nb1, scalar1=-1.0)

            # ================= r1p build =================
            r1p = stage.tile([C, 88, ST], bf16, tag="stage")
            r1r = r1p
            nc.vector.memset(r1p, 0.0)

            def rel(dst, src):
                nc.scalar.activation(out=dst, in_=src, func=AF.Relu,
                                     bias=nb1, scale=rs1)

            rel(r1r[:, 3:87, 2:132], out1[:, 0:84, 0:130])
            rel(r1r[:, 2:3, 2:132], out1[:, 1:2, 0:130])
            rel(r1r[:, 3:87, 1:2], out1[:, 0:84, 1:2])
            rel(r1r[:, 2:3, 1:2], out1[:, 1:2, 1:2])
            rel(r1r[:, 3:87, 132:133], out1[:, 0:84, 128:129])
            rel(r1r[:, 2:3, 132:133], out1[:, 1:2, 128:129])
            if debug_taps:
                nc.sync.dma_start(out=taps["r1p"][:, :, :], in_=r1p)
            ckpt(3)

            # ================= conv2 =================
            w2_sb = load_weights(w2_d, 9, C)
            out2 = stage.tile([C, 84, ST], bf16, tag="stage")
            r1_f = flat(r1r)

            def conv2_tile(t):
                pt = cps.tile([C, N1], f32, tag="cps")
                ob = 3 * t * ST
                i = 0
                for di in range(3):
                    for dj in range(3):
                        off = (di + 1) * ST + (dj - 3)
                        nc.tensor.matmul(
                            out=pt[:, :], lhsT=w2_sb[:, di * 3 + dj, :],
                            rhs=r1_f[:, ob + off: ob + off + N1],
                            start=(i == 0), stop=(i == 8))
                        i += 1
                nc.scalar.copy(out=flat(out2)[:, ob:ob + N1], in_=pt[:, :])

            for t in range(22):
                conv2_tile(t)
            if stage_limit == 31:
                for t in range(22, 28):
                    conv2_tile(t)
                if debug_taps:
                    nc.sync.dma_start(out=taps["out2"][:, :, :], in_=out2)
            if stage_limit == 31:
                raise _StopBuild()
            # IN2 stats + dct-mean dot:  band rows 0..65, cols at slots 3..134
            nc.vector.memset(out2[:, 0:66, 0:3], 0.0)
            nc.vector.memset(out2[:, 0:66, 135:138], 0.0)
            mv2 = bn_flat_stats(out2, C, 0, 66 * ST)
            if stage_limit == 32:
                raise _StopBuild()
            acc = sm.tile([C, 22], f32, tag="dotacc")
            for j in range(22):
                wvt = wvp.tile([C, 3, H2], bf16, tag="wv")
                nc.sync.dma_start(
                    out=wvt, in_=wv_d[3 * j:3 * j + 3, :].partition_broadcast(C))
                scr = wvp.tile([C, 3, H2], f32, tag="scr")
                nc.vector.tensor_mul(out=scr,
                                     in0=out2[:, 3 * j:3 * j + 3, 3:135],
                                     in1=wvt)
                nc.vector.tensor_reduce(out=acc[:, j:j + 1], in_=scr,
                                        axis=AX.XY, op=AL.add)
            dotw = smc.tile([C, 1], f32, tag="dotw")
            nc.vector.tensor_reduce(out=dotw, in_=acc, axis=AX.X, op=AL.add)
            if stage_limit == 33:
                raise _StopBuild()
            pk2 = sums_from_mv(mv2, 66 * ST)
            pk2b = smc.tile([C, 3], f32, tag="pk2b")
            nc.vector.tensor_copy(out=pk2b[:, 0:2], in_=pk2)
            nc.vector.tensor_copy(out=pk2b[:, 2:3], in_=dotw)
            gl2 = all_reduce(pk2b, 3, PAIRS)
            if stage_limit == 34:
                raise _StopBuild()
            for t in range(22, 28):
                conv2_tile(t)
            mu2, rs2 = mu_rstd_from_sums(gl2, CNT2_TOT)
            if debug_taps:
                nc.sync.dma_start(out=taps["out2"][:, :, :], in_=out2)
            ckpt(4)
            if debug_taps:
                nc.sync.dma_start(out=taps["stats2"][:, :], in_=gl2)

            # ================= SFOM gate =================
            # m = rs2 * (dotw_glob - mu2 * S_w)
            m_sb = smc.tile([C, 1], f32, tag="m")
            nc.vector.tensor_scalar_mul(out=m_sb, in0=mu2, scalar1=-S_W)
            nc.vector.tensor_add(out=m_sb, in0=m_sb, in1=gl2[:, 2:3])
            nc.vector.tensor_mul(out=m_sb, in0=m_sb, in1=rs2)
            # gate = sigmoid(relu(m @ sa1) @ sa2)
            p_r = mps.tile([8, 1], f32, tag="mps")
            nc.tensor.matmul(out=p_r, lhsT=sa1_sb, rhs=m_sb, start=True, stop=True)
            relu_sb = smc.tile([8, 1], f32, tag="relu8")
            nc.scalar.activation(out=relu_sb, in_=p_r, func=AF.Relu,
                                 bias=0.0, scale=1.0)
            p_g = mps.tile([C, 1], f32, tag="mps")
            nc.tensor.matmul(out=p_g, lhsT=sa2_sb, rhs=relu_sb,
                             start=True, stop=True)
            gate = smc.tile([C, 1], f32, tag="gate")
            nc.scalar.activation(out=gate, in_=p_g, func=AF.Sigmoid,
                                 bias=0.0, scale=1.0)
            if debug_taps:
                nc.sync.dma_start(out=taps["gate"][:, :], in_=gate)
            ckpt(5)
            # s_sig = rs2 * (1+gate)/2
            ssig = smc.tile([C, 1], f32, tag="ssig")
            nc.vector.tensor_scalar(out=ssig, in0=gate, scalar1=0.5, scalar2=0.5,
                                    op0=AL.mult, op1=AL.add)
            nc.vector.tensor_mul(out=ssig, in0=ssig, in1=rs2)
            nbs = smc.tile([C, 1], f32, tag="nbs")     # -mu2*ssig
            nc.vector.tensor_mul(out=nbs, in0=mu2, in1=ssig)
            nc.vector.tensor_scalar_mul(out=nbs, in0=nbs, scalar1=-1.0)

            # ================= SFOM apply =================
            # o2 = (out2-mu2)*rs2 ; f = sigmoid(o2*g2')*o2  (slots +4 rows in f)
            o2 = stage.tile([C, 84, ST], bf16, tag="stage")
            nc.vector.tensor_scalar(out=o2[:, :, 3:135], in0=out2[:, :, 3:135],
                                    scalar1=mu2, scalar2=rs2,
                                    op0=AL.subtract, op1=AL.mult)
            ftile = fpool.tile([C, R_F + 5, ST], bf16, tag="f")
            fr = ftile
            nc.vector.memset(ftile, 0.0)
            # sig = Sigmoid(out2*ssig + nbs)  (== sigmoid(o2*g2'))
            nc.scalar.activation(out=fr[:, 4:88, 3:135], in_=out2[:, :, 3:135],
                                 func=AF.Sigmoid, bias=nbs, scale=ssig)
            nc.vector.tensor_mul(out=fr[:, 4:88, 3:135],
                                 in0=ftile[:, 4:88, 3:135], in1=o2[:, :, 3:135])
            if debug_taps:
                nc.sync.dma_start(out=taps["f"][:, :, :], in_=ftile)
            ckpt(6)

            # ================= SPEM layers =================
            # per layer: (cin_packed_src, R, co, mm plan, psum N, combine)
            def spem_layer(lidx, src_r, wtile, co, R, NP, mms, combine, ncopies,
                           copy_cp):
                """Emit one SPEM conv layer.

                mms: list of (slice_idx, beta) matmul descriptors (lhsT slice of
                     wtile, rhs offset); combine(pt, dst_flat, ob) drains psum.
                ncopies/copy_cp: shifted-copy count and partition width for
                     K-packing of the NEXT layer's input.
                """
                S = R + 5
                h = stage.tile([128 if (ncopies or co > 64) else co, S, ST],
                               bf16, tag="stage")
                hr = h
                nc.vector.memset(h, 0.0)
                hf = (flat(hr), flat(h))
                src_f = flat(src_r)
                ntiles = R // 3

                def conv_tile(t):
                    pt = cps.tile([mms_part, NP], f32, tag="cps")
                    ob = (4 + 3 * t) * ST
                    for i, (sl, beta) in enumerate(mms):
                        nc.tensor.matmul(
                            out=pt[:, :], lhsT=wtile[:, sl, :],
                            rhs=src_f[:, ob + beta: ob + beta + NP],
                            start=(i == 0), stop=(i == len(mms) - 1))
                    combine(pt, hf, ob)

                mms_part = 128
                for t in range(22):
                    conv_tile(t)
                nc.vector.memset(h[0:co, 4:70, 0:3], 0.0)
                nc.vector.memset(h[0:co, 4:70, 135:138], 0.0)
                mvb = bn_flat_stats(h, co, 4 * ST, 66 * ST)
                pkb = sums_from_mv(mvb, 66 * ST, p=co)
                pkb128 = smc.tile([C, 2], f32, tag=f"pkb128_{lidx}")
                if co < C:
                    nc.vector.memset(pkb128, 0.0)
                nc.vector.tensor_copy(out=pkb128[0:co, :], in_=pkb)
                glb = all_reduce(pkb128, 2, ALL8)
                for t in range(22, ntiles):
                    conv_tile(t)
                mub, rsb = mu_rstd_from_sums(glb[0:co, :], CNTB_TOT, p=co)
                # scale = gamma*rstd ; bias = beta - mu*scale
                sc = smc.tile([co, 1], f32, tag=f"sc{lidx}")
                bi = smc.tile([co, 1], f32, tag=f"bi{lidx}")
                nc.vector.tensor_mul(out=sc, in0=gb_sb[0:co, lidx, 0:1], in1=rsb)
                nc.vector.tensor_mul(out=bi, in0=mub, in1=sc)
                nc.vector.tensor_sub(out=bi, in0=gb_sb[0:co, lidx, 1:2], in1=bi)
                nc.scalar.activation(out=hr[0:co, 4:4 + R, 3:135],
                                     in_=h[0:co, 4:4 + R, 3:135],
                                     func=AF.Relu, bias=bi, scale=sc)
                if R > 66:
                    nc.vector.memset(h[0:co, 70:4 + R, 0:3], 0.0)
                    nc.vector.memset(h[0:co, 70:4 + R, 135:138], 0.0)
                for g in range(1, ncopies + 1):
                    nc.vector.tensor_copy(
                        out=hr[g * copy_cp:(g + 1) * copy_cp, :, 0:ST - g],
                        in_=hr[0:copy_cp, :, g:ST])
                if debug_taps:
                    tp = taps[f"h{lidx+1}"]
                    nc.sync.dma_start(out=tp[:, :, :], in_=h[0:tp.shape[0], :, :])
                return hr

            def drain_act(pt, hf, ob):
                nc.scalar.copy(out=hf[0][:, ob:ob + 414], in_=pt[:, 0:414])

            def mk_combine(groups, cp):
                """groups: list of (psum partition group idx, col shift).
                DVE reads at most one PSUM operand: copy then accumulate."""
                def comb(pt, hf, ob):
                    hfr, hf32 = hf
                    g0, s0 = groups[0]
                    nc.vector.tensor_copy(
                        out=hfr[0:cp, ob:ob + 414],
                        in_=pt[g0 * cp:(g0 + 1) * cp, s0:s0 + 414])
                    for g, s in groups[1:]:
                        nc.vector.tensor_add(
                            out=hfr[0:cp, ob:ob + 414],
                            in0=hf32[0:cp, ob:ob + 414],
                            in1=pt[g * cp:(g + 1) * cp, s:s + 414])
                return comb

            # L1: 128->32, Mpack4: psum[g*32+co] <-> out[n-g]
            s1_sb = load_weights(s_d[0], 14, 128)
            mms1 = [(di * 2 + s, (di - 3) * ST + 4 * s - 3)
                    for di in range(7) for s in range(2)]
            h1 = spem_layer(0, fr, s1_sb, 32, R_H[0], 418, mms1,
                            mk_combine([(0, 0), (1, 1), (2, 2), (3, 3)], 32),
                            3, 32)
            # L2: 32->64, Kpack4 + Mpack2(supergroups +4): psum[G*64+co]<->out[n-4G]
            ckpt(7)
            s2_sb = load_weights(s_d[1], 7, 128)
            mms2 = [(di, (di - 3) * ST - 3) for di in range(7)]
            h2 = spem_layer(1, h1, s2_sb, 64, R_H[1], 418, mms2,
                            mk_combine([(0, 0), (1, 4)], 64), 1, 64)
            # L3: 64->128, Kpack2: 4 dj-groups
            ckpt(8)
            s3_sb = load_weights(s_d[2], 28, 128)
            mms3 = [(di * 4 + g, (di - 3) * ST + 2 * g - 3)
                    for di in range(7) for g in range(4)]
            h3 = spem_layer(2, h2, s3_sb, 128, R_H[2], 414, mms3, drain_act, 0, 0)
            # L4: 128->128 plain
            ckpt(9)
            s4_sb = load_weights(s_d[3], 49, 128)
            mms4 = [(di * 7 + dj, (di - 3) * ST + dj - 3)
                    for di in range(7) for dj in range(7)]
            h4 = spem_layer(3, h3, s4_sb, 128, R_H[3], 414, mms4, drain_act, 0, 0)
            # L5: 128->64, Mpack2: psum[g*64+co] <-> out[n-g]
            ckpt(10)
            s5_sb = load_weights(s_d[4], 28, 128)
            mms5 = [(di * 4 + st, (di - 3) * ST + 2 * st - 3)
                    for di in range(7) for st in range(4)]
            h5 = spem_layer(4, h4, s5_sb, 64, R_H[4], 416, mms5,
                            mk_combine([(0, 0), (1, 1)], 64), 1, 64)
            # L6: 64->32, Kpack2 + Mpack2: psum[G*32+co] <-> out[n-2G]
            ckpt(11)
            s6_sb = load_weights(s_d[5], 14, 128)
            mms6 = [(di * 2 + T, (di - 3) * ST + 4 * T - 3)
                    for di in range(7) for T in range(2)]
            h6 = spem_layer(5, h5, s6_sb, 32, R_H[5], 416, mms6,
                            mk_combine([(0, 0), (1, 2)], 32), 0, 0)

            ckpt(12)
            # ================= 1x1 conv + finale =================
            h6f32 = flat(h6)
            f_f = flat(ftile)
            for t in range(22):
                ob = (4 + 3 * t) * ST
                p7 = mps.tile([1, 414], f32, tag="mps")
                nc.tensor.matmul(out=p7, lhsT=cw_sb,
                                 rhs=h6f32[:, ob:ob + 414], start=True, stop=True)
                sa_c = sap.tile([1, 414], bf16, tag="sa")
                nc.scalar.activation(out=sa_c, in_=p7,
                                     func=AF.Sigmoid, bias=cb_sb[0:1, 0:1],
                                     scale=1.0)
                prep = mps.tile([128, 414], f32, tag="mps")
                nc.tensor.matmul(out=prep, lhsT=onesr_sb,
                                 rhs=sa_c, start=True, stop=True)
                ot = otp.tile([C, 414], f32, tag="ot")
                nc.vector.tensor_mul(out=ot, in0=prep, in1=f_f[:, ob:ob + 414])
                otv = ot.rearrange("p (r c) -> p r c", c=ST)
                nc.sync.dma_start(out=out_d[:, 3 * t:3 * t + 3, :],
                                  in_=otv[:, :, 3:135])


        try:
            _build_body()
        except _StopBuild:
            pass
        for p in [drp, mps, cps, sap, otp, wvp, smc, sm, cons, wts, fpool,
                  stage]:
            p.release()

    nc.compile()
    _PROGRAM_CACHE[key] = (nc, taps)
    return nc, taps


# ----------------------------------------------------------------------------
# host-side packing
# ----------------------------------------------------------------------------

def _pack_core_inputs(inputs, core):
    b, half = core // 2, core % 2
    flip = (half == 1)

    def fd(w):          # flip di (axis 2) of [co, ci, kh, kw]
        return w[:, :, ::-1, :] if flip else w

    x = inputs['x'][b]
    if flip:
        x = x[:, ::-1, :]
    xs = np.ascontiguousarray(x[:, 0:XROWS, :]).astype(BF16)

    w1 = fd(inputs['conv1_w'])
    w2 = fd(inputs['conv2_w'])
    w1p = np.ascontiguousarray(
        np.transpose(w1, (2, 3, 1, 0)).reshape(9, C, C).transpose(1, 0, 2),
        np.float32)
    w2p = np.ascontiguousarray(
        np.transpose(w2, (2, 3, 1, 0)).reshape(9, C, C).transpose(1, 0, 2),
        np.float32)

    dws = [fd(inputs[f'dw{i}']) for i in range(1, 7)]

    def lhsT(w, di, dj):
        return w[:, :, di, dj].T        # [ci, co]

    # s1 [C, 14, 128]: idx di*2+s ; cols g*32+co = dj=4s+g
    s1 = np.zeros((C, 14, 128), np.float32)
    for di in range(7):
        for s in range(2):
            for g in range(4):
                dj = 4 * s + g
                if dj < 7:
                    s1[:, di * 2 + s, 32 * g:32 * g + 32] = lhsT(dws[0], di, dj)
    # s2 [C, 7, 128]: idx di ; rows s*32+ci ; cols G*64+co = dj=4G+s
    s2 = np.zeros((C, 7, 128), np.float32)
    for di in range(7):
        for s in range(4):
            for G in range(2):
                dj = 4 * G + s
                if dj < 7:
                    s2[32 * s:32 * s + 32, di, 64 * G:64 * G + 64] = \
                        lhsT(dws[1], di, dj)
    # s3 [C, 28, 128]: idx di*4+grp ; rows s*64+ci ; dj=2grp+s
    s3 = np.zeros((C, 28, 128), np.float32)
    for di in range(7):
        for grp in range(4):
            for s in range(2):
                dj = 2 * grp + s
                if dj < 7:
                    s3[64 * s:64 * s + 64, di * 4 + grp, :] = \
                        lhsT(dws[2], di, dj)
    # s4 [C, 49, 128]
    s4 = np.zeros((C, 49, 128), np.float32)
    for di in range(7):
        for dj in range(7):
            s4[:, di * 7 + dj, :] = lhsT(dws[3], di, dj)
    # s5 [C, 28, 128]: idx di*4+st ; cols g*64+co = dj=2st+g
    s5 = np.zeros((C, 28, 128), np.float32)
    for di in range(7):
        for st in range(4):
            for g in range(2):
                dj = 2 * st + g
                if dj < 7:
                    s5[:, di * 4 + st, 64 * g:64 * g + 64] = \
                        lhsT(dws[4], di, dj)
    # s6 [C, 14, 128]: idx di*2+T ; rows s*64+ci ; cols G*32+co = dj=4T+2G+s
    # (cols 64..127 stay zero: M padded to 128 for the f32r col_grp rule)
    s6 = np.zeros((C, 14, 128), np.float32)
    for di in range(7):
        for T in range(2):
            for G in range(2):
                for s in range(2):
                    dj = 4 * T + 2 * G + s
                    if dj < 7:
                        s6[64 * s:64 * s + 64, di * 2 + T,
                           32 * G:32 * G + 32] = lhsT(dws[5], di, dj)

    wvec = dct_mean_weights().reshape(H2, H2)
    if flip:
        wv = np.ascontiguousarray(wvec[::-1, :][0:HB]).astype(BF16)
    else:
        wv = np.ascontiguousarray(wvec[0:HB]).astype(BF16)

    gb = np.zeros((6, 2, C), np.float32)
    for k in range(6):
        g = inputs[f'bg{k+1}']
        bb = inputs[f'bb{k+1}']
        gb[k, 0, :len(g)] = g
        gb[k, 1, :len(bb)] = bb

    return {
        'xs': xs, 'w1': w1p.astype(BF16), 'w2c': w2p.astype(BF16),
        's1': s1.astype(BF16), 's2': s2.astype(BF16), 's3': s3.astype(BF16),
        's4': s4.astype(BF16), 's5': s5.astype(BF16), 's6': s6.astype(BF16),
        'cw': np.ascontiguousarray(inputs['spem_cw'][0, :, 0, 0][:, None]
                                   ).astype(BF16),
        'cb': np.asarray(inputs['spem_cb'], np.float32).reshape(1, 1),
        'sa1t': np.ascontiguousarray(inputs['sa_w1'].T, np.float32),
        'sa2t': np.ascontiguousarray(inputs['sa_w2'].T, np.float32),
        'gb': gb, 'wv': wv,
    }


def run_cores(inputs, trace=False, debug_taps=False, stage_limit=99):
    _install_ntff_hook()
    from concourse.bass_utils import run_bass_kernel_spmd
    nc, taps = build_program(debug_taps=debug_taps, stage_limit=stage_limit)
    in_maps = [_pack_core_inputs(inputs, c) for c in range(NCORES)]
    res = run_bass_kernel_spmd(nc, in_maps, list(range(NCORES)), trace=trace)
    return res


def kernel(**inputs):
    res = run_cores(inputs)
    full = np.empty((B, C, H2, H2), np.float32)
    for b in range(B):
        full[b, :, 0:HB, :] = res.results[2 * b]["out"]
        full[b, :, HB:H2, :] = res.results[2 * b + 1]["out"][:, ::-1, :]
    return full

